# revision 7
# baseline (speedup 1.0000x reference)
"""GaussianImage (Cholesky) renderer on 8 trn2 NeuronCores.

Strategy: tile-parallel over the pixel grid (sharding_hint alternative 2).
The 256x256 image is cut into 32x32-pixel tiles (64/frame, 128 total for
T=2).  The host bins gaussians to tiles (pure routing: bbox intersect via a
conservative support radius; outside it exp(-sigma) underflows to 0 in
fp32), pads each tile's gaussian list to 128 slots, and hands every core 16
tile-entries with slot-ordered copies of the RAW inputs.  All math runs on
device:

  per gaussian slot : tanh / sigmoid / conic / quadratic-basis coeffs
  per tile          : sigma = lhsT(6,128)^T @ basis(6,1024)   [TensorE fp32]
                      alpha = Exp(-sigma)                     [ScalarE]
                      img   = w(128,3)^T @ alpha(128,1024)    [TensorE fp32]
                      out   = clamp(img, 0, 1)                [VectorE, fused]

Each pixel is owned by exactly one tile -> no cross-core reduction.
"""

import os
import numpy as np

T, N, H, W = 2, 512, 256, 256
TILE = 32
NT = H // TILE          # 8 tiles per axis
N_CORES = 8
SLOTS = 128
PIX = TILE * TILE       # 1024
SIGMA_CUT = 100.0       # exp(-100) ~ 4e-44: below fp32 denormal resolution

_CACHE = {}


def _build_nc(E, mm2_dtype_name="float32"):
    import concourse.bass as bass
    import concourse.mybir as mybir
    from concourse.tile import TileContext
    import bass_rust

    f32 = mybir.dt.float32
    Alu = mybir.AluOpType
    Act = mybir.ActivationFunctionType

    nc = bass.Bass("TRN2")
    params = nc.dram_tensor("params", [SLOTS, E * 12], f32, kind="ExternalInput")
    basis = nc.dram_tensor("basis", [6, PIX], f32, kind="ExternalInput")
    ident = nc.dram_tensor("ident", [SLOTS, SLOTS], f32, kind="ExternalInput")
    out = nc.dram_tensor("out", [3, E * PIX], f32, kind="ExternalOutput")

    with TileContext(nc) as tc:
        with tc.tile_pool(name="const", bufs=1) as cpool, \
             tc.tile_pool(name="work", bufs=3) as wpool, \
             tc.tile_pool(name="ps_sig", bufs=2, space="PSUM") as ps_sig_pool, \
             tc.tile_pool(name="ps_img", bufs=2, space="PSUM") as ps_img_pool:

            p3 = cpool.tile([SLOTS, E, 12], f32, tag="params")
            bt = cpool.tile([6, PIX], f32, tag="basis")
            it = cpool.tile([SLOTS, SLOTS], f32, tag="ident")
            nc.gpsimd.dma_start(out=p3, in_=params[:].rearrange("p (e k) -> p e k", k=12))
            nc.gpsimd.dma_start(out=bt, in_=basis[:])
            nc.gpsimd.dma_start(out=it, in_=ident[:])

            def sc(tag):
                return cpool.tile([SLOTS, E], f32, tag=tag, name=tag)

            V = nc.vector
            S = nc.scalar

            # --- per-slot prep (all (128,E)) ---
            mx, my = sc("mx"), sc("my")
            S.activation(mx, p3[:, :, 0], Act.Tanh)
            S.activation(my, p3[:, :, 1], Act.Tanh)
            cxp, cyp = sc("cxp"), sc("cyp")
            V.tensor_scalar(out=cxp, in0=mx, scalar1=0.5 * W, scalar2=0.5 * W,
                            op0=Alu.mult, op1=Alu.add)
            V.tensor_scalar(out=cyp, in0=my, scalar1=0.5 * H, scalar2=0.5 * H,
                            op0=Alu.mult, op1=Alu.add)
            ex, ey = sc("ex"), sc("ey")
            V.tensor_sub(out=ex, in0=cxp, in1=p3[:, :, 9])
            V.tensor_sub(out=ey, in0=cyp, in1=p3[:, :, 10])

            a0, a2 = sc("a0"), sc("a2")
            V.tensor_scalar_add(out=a0, in0=p3[:, :, 2], scalar1=0.5)
            V.tensor_scalar_add(out=a2, in0=p3[:, :, 4], scalar1=0.5)
            a1 = p3[:, :, 3]
            t0, t1, t2, t3 = sc("t0"), sc("t1"), sc("t2"), sc("t3")
            V.tensor_mul(out=t0, in0=a0, in1=a0)
            V.tensor_mul(out=t1, in0=a0, in1=a1)
            V.tensor_mul(out=t2, in0=a1, in1=a1)
            V.tensor_mul(out=t3, in0=a2, in1=a2)
            syy = sc("syy")
            V.tensor_add(out=syy, in0=t2, in1=t3)
            u, v, det, rdet = sc("u"), sc("v"), sc("det"), sc("rdet")
            V.tensor_mul(out=u, in0=t0, in1=syy)
            V.tensor_mul(out=v, in0=t1, in1=t1)
            V.tensor_sub(out=det, in0=u, in1=v)
            V.reciprocal(out=rdet, in_=det)
            ca, cbn, cc = sc("ca"), sc("cbn"), sc("cc")
            V.tensor_mul(out=ca, in0=syy, in1=rdet)   # conic a
            V.tensor_mul(out=cbn, in0=t1, in1=rdet)   # -conic b
            V.tensor_mul(out=cc, in0=t0, in1=rdet)    # conic c

            ct = cpool.tile([SLOTS, E, 6], f32, tag="coef")
            V.tensor_scalar_mul(out=ct[:, :, 0], in0=ca, scalar1=0.5)
            V.tensor_scalar_mul(out=ct[:, :, 1], in0=cbn, scalar1=-1.0)
            V.tensor_scalar_mul(out=ct[:, :, 2], in0=cc, scalar1=0.5)
            m1, m2 = sc("m1"), sc("m2")
            V.tensor_mul(out=m1, in0=ca, in1=ex)
            V.tensor_mul(out=m2, in0=cbn, in1=ey)
            V.tensor_sub(out=ct[:, :, 3], in0=m2, in1=m1)    # -(ca*ex + cb*ey)
            m3, m4 = sc("m3"), sc("m4")
            V.tensor_mul(out=m3, in0=cc, in1=ey)
            V.tensor_mul(out=m4, in0=cbn, in1=ex)
            V.tensor_sub(out=ct[:, :, 4], in0=m4, in1=m3)    # -(cc*ey + cb*ex)
            exx, exy, eyy = sc("exx"), sc("exy"), sc("eyy")
            V.tensor_mul(out=exx, in0=ex, in1=ex)
            V.tensor_mul(out=exy, in0=ex, in1=ey)
            V.tensor_mul(out=eyy, in0=ey, in1=ey)
            p1, p2, p3b, q = sc("p1"), sc("p2"), sc("p3b"), sc("q")
            V.tensor_mul(out=p1, in0=ct[:, :, 0], in1=exx)
            V.tensor_mul(out=p2, in0=cbn, in1=exy)
            V.tensor_mul(out=p3b, in0=ct[:, :, 2], in1=eyy)
            V.tensor_sub(out=q, in0=p1, in1=p2)
            V.tensor_add(out=ct[:, :, 5], in0=q, in1=p3b)

            osg = sc("osg")
            S.activation(osg, p3[:, :, 5], Act.Sigmoid)
            wt = cpool.tile([SLOTS, E, 3], f32, tag="w")
            S.activation(wt, p3[:, :, 6:9], Act.Sigmoid)
            f32r = mybir.dt.float32r
            wtr = cpool.tile([SLOTS, E, 3], f32r, tag="wr")
            for k in range(3):
                V.tensor_mul(out=wtr[:, :, k], in0=wt[:, :, k], in1=osg)

            # --- transpose coeffs per entry: (128,6) -> (6,128) via TensorE ---
            lhsT = cpool.tile([6, E * SLOTS], f32, tag="lhsT")
            for e in range(E):
                tp = ps_img_pool.tile([6, SLOTS], f32, tag="img", name=f"tp{e}")
                nc.tensor.transpose(tp, ct[:, e, :], it)
                V.tensor_copy(out=lhsT[:, e * SLOTS:(e + 1) * SLOTS], in_=tp)

            st = cpool.tile([3, E * PIX], f32, tag="stage")

            # --- hot loop ---
            for e in range(E):
                sig = ps_sig_pool.tile([SLOTS, PIX], f32, tag="sig")
                lh = lhsT[:, e * SLOTS:(e + 1) * SLOTS]
                nc.tensor.matmul(sig[:, 0:512], lh, bt[:, 0:512], start=True, stop=True)
                nc.tensor.matmul(sig[:, 512:1024], lh, bt[:, 512:1024], start=True, stop=True)
                alpha = wpool.tile([SLOTS, PIX], f32r, tag="alpha")
                S.activation(alpha, sig, Act.Exp, scale=-1.0)
                img = ps_img_pool.tile([3, PIX], f32, tag="img")
                wre = wtr[:, e, :]
                nc.tensor.matmul(img[:, 0:512], wre, alpha[:, 0:512], start=True, stop=True)
                nc.tensor.matmul(img[:, 512:1024], wre, alpha[:, 512:1024], start=True, stop=True)
                V.tensor_scalar(out=st[:, e * PIX:(e + 1) * PIX], in0=img,
                                scalar1=0.0, scalar2=1.0, op0=Alu.max, op1=Alu.min)

            nc.gpsimd.dma_start(out=out[:], in_=st)

    bass_rust.generate_event_semaphores(nc)
    return nc


def _bin_entries(xyz, cholesky):
    """Host-side routing: which gaussians overlap which 32x32 tile."""
    means = np.tanh(xyz.astype(np.float64))
    cx = 0.5 * W * (means[..., 0] + 1.0)
    cy = 0.5 * H * (means[..., 1] + 1.0)
    chol = cholesky.astype(np.float64) + np.array([0.5, 0.0, 0.5])
    l0, l1, l2 = chol[..., 0], chol[..., 1], chol[..., 2]
    sxx, sxy, syy = l0 * l0, l0 * l1, l1 * l1 + l2 * l2
    tr, det = sxx + syy, sxx * syy - sxy * sxy
    lam = tr / 2 + np.sqrt(np.maximum(tr * tr / 4 - det, 0.0))
    r = np.sqrt(2.0 * SIGMA_CUT * np.maximum(lam, 0.0)) + 1.0

    entries = []  # (frame, ty, tx, index-list)
    for t in range(T):
        x0 = np.clip(((cx[t] - r[t]) // TILE).astype(int), 0, NT - 1)
        x1 = np.clip(((cx[t] + r[t]) // TILE).astype(int), 0, NT - 1)
        y0 = np.clip(((cy[t] - r[t]) // TILE).astype(int), 0, NT - 1)
        y1 = np.clip(((cy[t] + r[t]) // TILE).astype(int), 0, NT - 1)
        buckets = [[[] for _ in range(NT)] for _ in range(NT)]
        for n in range(N):
            for ty in range(y0[n], y1[n] + 1):
                for tx in range(x0[n], x1[n] + 1):
                    buckets[ty][tx].append(n)
        for ty in range(NT):
            for tx in range(NT):
                assert len(buckets[ty][tx]) <= SLOTS, "tile overflow: >128 gaussians"
                entries.append((t, ty, tx, buckets[ty][tx]))
    return entries


def _ensure_ntff_hook():
    """Provide antenv.axon_hooks (missing in this image) so trace=True works."""
    import sys, types, ctypes, contextlib
    if "antenv.axon_hooks" in sys.modules:
        return
    so_path = "/opt/axon/libaxon_pjrt.so"
    if not os.path.exists(so_path):
        return
    lib = ctypes.CDLL(so_path)
    if not hasattr(lib, "axon_start_nrt_profile"):
        return
    lib.axon_start_nrt_profile.argtypes = [ctypes.POINTER(ctypes.c_int64), ctypes.c_size_t]
    lib.axon_start_nrt_profile.restype = ctypes.c_int64
    lib.axon_stop_nrt_profile.argtypes = [ctypes.c_char_p]
    lib.axon_stop_nrt_profile.restype = ctypes.c_int64

    @contextlib.contextmanager
    def _hook(output_dir, device_ids):
        import jax
        jax.devices()
        if device_ids:
            ids = (ctypes.c_int64 * len(device_ids))(*device_ids)
            rc = lib.axon_start_nrt_profile(ids, len(device_ids))
        else:
            rc = lib.axon_start_nrt_profile(None, 0)
        if rc != 0:
            raise RuntimeError(f"axon_start_nrt_profile rc={rc}")
        try:
            yield
        finally:
            n = lib.axon_stop_nrt_profile(str(output_dir).encode())
            print(f"profile: {n} file(s) written to {output_dir}")

    mod = types.ModuleType("antenv.axon_hooks")
    mod.get_axon_ntff_profile_hook = lambda: _hook
    mod.set_axon_ntff_profile_hook = lambda h: None
    sys.modules["antenv.axon_hooks"] = mod


def kernel(xyz, cholesky, opacity, features_dc):
    from concourse import bass_utils

    xyz = np.asarray(xyz, np.float32)
    cholesky = np.asarray(cholesky, np.float32)
    opacity = np.asarray(opacity, np.float32)
    features_dc = np.asarray(features_dc, np.float32)

    entries = _bin_entries(xyz, cholesky)
    E = (len(entries) + N_CORES - 1) // N_CORES

    # per-core packed params: (128, E, 12) -> flat (128, E*12)
    in_maps = []
    gx = np.arange(PIX, dtype=np.float32) % TILE
    gy = np.arange(PIX, dtype=np.float32) // TILE
    basis = np.stack([gx * gx, gx * gy, gy * gy, gx, gy, np.ones(PIX, np.float32)]).astype(np.float32)
    ident = np.eye(SLOTS, dtype=np.float32)
    for c in range(N_CORES):
        pm = np.zeros((SLOTS, E, 12), np.float32)
        pm[:, :, 5] = -100.0  # dummy slots: sigmoid(opacity) ~ 0
        for ei in range(E):
            k = c * E + ei
            if k >= len(entries):
                continue
            t, ty, tx, idxs = entries[k]
            ns = len(idxs)
            if ns:
                idxs = np.asarray(idxs)
                pm[:ns, ei, 0:2] = xyz[t, idxs]
                pm[:ns, ei, 2:5] = cholesky[t, idxs]
                pm[:ns, ei, 5] = opacity[idxs, 0]
                pm[:ns, ei, 6:9] = features_dc[idxs]
            pm[:, ei, 9] = tx * TILE
            pm[:, ei, 10] = ty * TILE
        in_maps.append({"params": pm.reshape(SLOTS, E * 12),
                        "basis": basis, "ident": ident})

    if E not in _CACHE:
        _CACHE[E] = _build_nc(E)
    nc = _CACHE[E]

    trace = bool(int(os.environ.get("GS_TRACE", "0")))
    if trace:
        _ensure_ntff_hook()
    res = bass_utils.run_bass_kernel_spmd(
        nc, in_maps, core_ids=list(range(N_CORES)), trace=trace)
    kernel.last_result = res

    img = np.zeros((T, 3, H, W), np.float32)
    for c in range(N_CORES):
        o = res.results[c]["out"].reshape(3, E, TILE, TILE)
        for ei in range(E):
            k = c * E + ei
            if k >= len(entries):
                continue
            t, ty, tx, _ = entries[k]
            img[t, :, ty * TILE:(ty + 1) * TILE, tx * TILE:(tx + 1) * TILE] = o[:, ei]
    return img


# revision 12
# speedup vs baseline: 1.1256x; 1.1256x over previous
"""GaussianImage (Cholesky) renderer on 8 trn2 NeuronCores.

Strategy: tile-parallel over the pixel grid (sharding_hint alternative 2).
The 256x256 image is cut into 32x32-pixel tiles (64/frame, 128 total for
T=2).  The host bins gaussians to tiles (pure routing: bbox intersect via a
conservative support radius; outside it exp(-sigma) underflows to 0 in
fp32), pads each tile's gaussian list to 128 slots, and hands every core 16
tile-entries with slot-ordered copies of the RAW inputs.  All math runs on
device:

  per gaussian slot : tanh / sigmoid / conic / quadratic-basis coeffs
  per tile          : sigma = lhsT(6,128)^T @ basis(6,1024)   [TensorE fp32]
                      alpha = Exp(-sigma)                     [ScalarE]
                      img   = w(128,3)^T @ alpha(128,1024)    [TensorE fp32]
                      out   = clamp(img, 0, 1)                [VectorE, fused]

Each pixel is owned by exactly one tile -> no cross-core reduction.
"""

import os
import numpy as np

T, N, H, W = 2, 512, 256, 256
TILE = 32
NT = H // TILE          # 8 tiles per axis
N_CORES = 8
SLOTS = 128
PIX = TILE * TILE       # 1024
SIGMA_CUT = 100.0       # exp(-100) ~ 4e-44: below fp32 denormal resolution

_CACHE = {}


def _build_nc(E, mm2_dtype_name="float32"):
    import concourse.bass as bass
    import concourse.mybir as mybir
    from concourse.tile import TileContext
    import bass_rust

    f32 = mybir.dt.float32
    Alu = mybir.AluOpType
    Act = mybir.ActivationFunctionType

    nc = bass.Bass("TRN2")
    params = nc.dram_tensor("params", [SLOTS, E * 12], f32, kind="ExternalInput")
    basis = nc.dram_tensor("basis", [6, PIX], f32, kind="ExternalInput")
    ident = nc.dram_tensor("ident", [SLOTS, SLOTS], f32, kind="ExternalInput")
    out = nc.dram_tensor("out", [3, E * PIX], f32, kind="ExternalOutput")

    with TileContext(nc) as tc:
        with tc.tile_pool(name="const", bufs=1) as cpool, \
             tc.tile_pool(name="work", bufs=3) as wpool, \
             tc.tile_pool(name="ps_sig", bufs=2, space="PSUM") as ps_sig_pool, \
             tc.tile_pool(name="ps_img", bufs=2, space="PSUM") as ps_img_pool:

            p3 = cpool.tile([SLOTS, E, 12], f32, tag="params")
            bt = cpool.tile([6, PIX], f32, tag="basis")
            it = cpool.tile([SLOTS, SLOTS], f32, tag="ident")
            nc.gpsimd.dma_start(out=p3, in_=params[:].rearrange("p (e k) -> p e k", k=12))
            nc.gpsimd.dma_start(out=bt, in_=basis[:])
            nc.gpsimd.dma_start(out=it, in_=ident[:])

            def sc(tag):
                return cpool.tile([SLOTS, E], f32, tag=tag, name=tag)

            V = nc.vector
            S = nc.scalar

            # --- per-slot prep (all (128,E)) ---
            mx, my = sc("mx"), sc("my")
            S.activation(mx, p3[:, :, 0], Act.Tanh)
            S.activation(my, p3[:, :, 1], Act.Tanh)
            cxp, cyp = sc("cxp"), sc("cyp")
            V.tensor_scalar(out=cxp, in0=mx, scalar1=0.5 * W, scalar2=0.5 * W,
                            op0=Alu.mult, op1=Alu.add)
            V.tensor_scalar(out=cyp, in0=my, scalar1=0.5 * H, scalar2=0.5 * H,
                            op0=Alu.mult, op1=Alu.add)
            ex, ey = sc("ex"), sc("ey")
            V.tensor_sub(out=ex, in0=cxp, in1=p3[:, :, 9])
            V.tensor_sub(out=ey, in0=cyp, in1=p3[:, :, 10])

            a0, a2 = sc("a0"), sc("a2")
            V.tensor_scalar_add(out=a0, in0=p3[:, :, 2], scalar1=0.5)
            V.tensor_scalar_add(out=a2, in0=p3[:, :, 4], scalar1=0.5)
            a1 = p3[:, :, 3]
            t0, t1, t2, t3 = sc("t0"), sc("t1"), sc("t2"), sc("t3")
            V.tensor_mul(out=t0, in0=a0, in1=a0)
            V.tensor_mul(out=t1, in0=a0, in1=a1)
            V.tensor_mul(out=t2, in0=a1, in1=a1)
            V.tensor_mul(out=t3, in0=a2, in1=a2)
            syy = sc("syy")
            V.tensor_add(out=syy, in0=t2, in1=t3)
            u, v, det, rdet = sc("u"), sc("v"), sc("det"), sc("rdet")
            V.tensor_mul(out=u, in0=t0, in1=syy)
            V.tensor_mul(out=v, in0=t1, in1=t1)
            V.tensor_sub(out=det, in0=u, in1=v)
            V.reciprocal(out=rdet, in_=det)
            ca, cbn, cc = sc("ca"), sc("cbn"), sc("cc")
            V.tensor_mul(out=ca, in0=syy, in1=rdet)   # conic a
            V.tensor_mul(out=cbn, in0=t1, in1=rdet)   # -conic b
            V.tensor_mul(out=cc, in0=t0, in1=rdet)    # conic c

            ct = cpool.tile([SLOTS, E, 6], f32, tag="coef")
            V.tensor_scalar_mul(out=ct[:, :, 0], in0=ca, scalar1=0.5)
            V.tensor_scalar_mul(out=ct[:, :, 1], in0=cbn, scalar1=-1.0)
            V.tensor_scalar_mul(out=ct[:, :, 2], in0=cc, scalar1=0.5)
            m1, m2 = sc("m1"), sc("m2")
            V.tensor_mul(out=m1, in0=ca, in1=ex)
            V.tensor_mul(out=m2, in0=cbn, in1=ey)
            V.tensor_sub(out=ct[:, :, 3], in0=m2, in1=m1)    # -(ca*ex + cb*ey)
            m3, m4 = sc("m3"), sc("m4")
            V.tensor_mul(out=m3, in0=cc, in1=ey)
            V.tensor_mul(out=m4, in0=cbn, in1=ex)
            V.tensor_sub(out=ct[:, :, 4], in0=m4, in1=m3)    # -(cc*ey + cb*ex)
            exx, exy, eyy = sc("exx"), sc("exy"), sc("eyy")
            V.tensor_mul(out=exx, in0=ex, in1=ex)
            V.tensor_mul(out=exy, in0=ex, in1=ey)
            V.tensor_mul(out=eyy, in0=ey, in1=ey)
            p1, p2, p3b, q = sc("p1"), sc("p2"), sc("p3b"), sc("q")
            V.tensor_mul(out=p1, in0=ct[:, :, 0], in1=exx)
            V.tensor_mul(out=p2, in0=cbn, in1=exy)
            V.tensor_mul(out=p3b, in0=ct[:, :, 2], in1=eyy)
            V.tensor_sub(out=q, in0=p1, in1=p2)
            V.tensor_add(out=ct[:, :, 5], in0=q, in1=p3b)

            osg = sc("osg")
            S.activation(osg, p3[:, :, 5], Act.Sigmoid)
            wt = cpool.tile([SLOTS, E, 3], f32, tag="w")
            S.activation(wt, p3[:, :, 6:9], Act.Sigmoid)
            f32r = mybir.dt.float32r
            wtr = cpool.tile([SLOTS, E, 3], f32r, tag="wr")
            for k in range(3):
                V.tensor_mul(out=wtr[:, :, k], in0=wt[:, :, k], in1=osg)

            # --- transpose coeffs: one PE transpose (128,6E)->(6E,128), then
            # slice per entry out of an SBUF copy ---
            tp = ps_img_pool.tile([6 * E, SLOTS], f32, tag="img", name="tp")
            nc.tensor.transpose(tp, ct.rearrange("p e k -> p (e k)"), it)
            tps = cpool.tile([6 * E, SLOTS], f32, tag="tps")
            V.tensor_copy(out=tps, in_=tp)
            lhsT = cpool.tile([6, E, SLOTS], f32, tag="lhsT")
            for e in range(E):
                nc.sync.dma_start(out=lhsT[:, e, :], in_=tps[6 * e:6 * e + 6, :])

            st = cpool.tile([3, E * PIX], f32, tag="stage")

            # --- hot loop ---
            for e in range(E):
                sig = ps_sig_pool.tile([SLOTS, PIX], f32, tag="sig")
                lh = lhsT[:, e, :]
                nc.tensor.matmul(sig[:, 0:512], lh, bt[:, 0:512], start=True, stop=True)
                nc.tensor.matmul(sig[:, 512:1024], lh, bt[:, 512:1024], start=True, stop=True)
                alpha = wpool.tile([SLOTS, PIX], f32r, tag="alpha")
                S.activation(alpha, sig, Act.Exp, scale=-1.0)
                img = ps_img_pool.tile([3, PIX], f32, tag="img")
                wre = wtr[:, e, :]
                nc.tensor.matmul(img[:, 0:512], wre, alpha[:, 0:512], start=True, stop=True)
                nc.tensor.matmul(img[:, 512:1024], wre, alpha[:, 512:1024], start=True, stop=True)
                V.tensor_scalar(out=st[:, e * PIX:(e + 1) * PIX], in0=img,
                                scalar1=0.0, scalar2=1.0, op0=Alu.max, op1=Alu.min)
                nc.sync.dma_start(out=out[:, e * PIX:(e + 1) * PIX],
                                  in_=st[:, e * PIX:(e + 1) * PIX])

    bass_rust.generate_event_semaphores(nc)
    return nc


def _bin_entries(xyz, cholesky):
    """Host-side routing: which gaussians overlap which 32x32 tile."""
    means = np.tanh(xyz.astype(np.float64))
    cx = 0.5 * W * (means[..., 0] + 1.0)
    cy = 0.5 * H * (means[..., 1] + 1.0)
    chol = cholesky.astype(np.float64) + np.array([0.5, 0.0, 0.5])
    l0, l1, l2 = chol[..., 0], chol[..., 1], chol[..., 2]
    sxx, sxy, syy = l0 * l0, l0 * l1, l1 * l1 + l2 * l2
    tr, det = sxx + syy, sxx * syy - sxy * sxy
    lam = tr / 2 + np.sqrt(np.maximum(tr * tr / 4 - det, 0.0))
    r = np.sqrt(2.0 * SIGMA_CUT * np.maximum(lam, 0.0)) + 1.0

    entries = []  # (frame, ty, tx, index-list)
    for t in range(T):
        x0 = np.clip(((cx[t] - r[t]) // TILE).astype(int), 0, NT - 1)
        x1 = np.clip(((cx[t] + r[t]) // TILE).astype(int), 0, NT - 1)
        y0 = np.clip(((cy[t] - r[t]) // TILE).astype(int), 0, NT - 1)
        y1 = np.clip(((cy[t] + r[t]) // TILE).astype(int), 0, NT - 1)
        buckets = [[[] for _ in range(NT)] for _ in range(NT)]
        for n in range(N):
            for ty in range(y0[n], y1[n] + 1):
                for tx in range(x0[n], x1[n] + 1):
                    buckets[ty][tx].append(n)
        for ty in range(NT):
            for tx in range(NT):
                assert len(buckets[ty][tx]) <= SLOTS, "tile overflow: >128 gaussians"
                entries.append((t, ty, tx, buckets[ty][tx]))
    return entries


def _ensure_ntff_hook():
    """Provide antenv.axon_hooks (missing in this image) so trace=True works."""
    import sys, types, ctypes, contextlib
    if "antenv.axon_hooks" in sys.modules:
        return
    so_path = "/opt/axon/libaxon_pjrt.so"
    if not os.path.exists(so_path):
        return
    lib = ctypes.CDLL(so_path)
    if not hasattr(lib, "axon_start_nrt_profile"):
        return
    lib.axon_start_nrt_profile.argtypes = [ctypes.POINTER(ctypes.c_int64), ctypes.c_size_t]
    lib.axon_start_nrt_profile.restype = ctypes.c_int64
    lib.axon_stop_nrt_profile.argtypes = [ctypes.c_char_p]
    lib.axon_stop_nrt_profile.restype = ctypes.c_int64

    @contextlib.contextmanager
    def _hook(output_dir, device_ids):
        import jax
        jax.devices()
        if device_ids:
            ids = (ctypes.c_int64 * len(device_ids))(*device_ids)
            rc = lib.axon_start_nrt_profile(ids, len(device_ids))
        else:
            rc = lib.axon_start_nrt_profile(None, 0)
        if rc != 0:
            raise RuntimeError(f"axon_start_nrt_profile rc={rc}")
        try:
            yield
        finally:
            n = lib.axon_stop_nrt_profile(str(output_dir).encode())
            print(f"profile: {n} file(s) written to {output_dir}")

    mod = types.ModuleType("antenv.axon_hooks")
    mod.get_axon_ntff_profile_hook = lambda: _hook
    mod.set_axon_ntff_profile_hook = lambda h: None
    sys.modules["antenv.axon_hooks"] = mod


def kernel(xyz, cholesky, opacity, features_dc):
    from concourse import bass_utils

    xyz = np.asarray(xyz, np.float32)
    cholesky = np.asarray(cholesky, np.float32)
    opacity = np.asarray(opacity, np.float32)
    features_dc = np.asarray(features_dc, np.float32)

    entries = _bin_entries(xyz, cholesky)
    E = (len(entries) + N_CORES - 1) // N_CORES

    # per-core packed params: (128, E, 12) -> flat (128, E*12)
    in_maps = []
    gx = np.arange(PIX, dtype=np.float32) % TILE
    gy = np.arange(PIX, dtype=np.float32) // TILE
    basis = np.stack([gx * gx, gx * gy, gy * gy, gx, gy, np.ones(PIX, np.float32)]).astype(np.float32)
    ident = np.eye(SLOTS, dtype=np.float32)
    for c in range(N_CORES):
        pm = np.zeros((SLOTS, E, 12), np.float32)
        pm[:, :, 5] = -100.0  # dummy slots: sigmoid(opacity) ~ 0
        for ei in range(E):
            k = c * E + ei
            if k >= len(entries):
                continue
            t, ty, tx, idxs = entries[k]
            ns = len(idxs)
            if ns:
                idxs = np.asarray(idxs)
                pm[:ns, ei, 0:2] = xyz[t, idxs]
                pm[:ns, ei, 2:5] = cholesky[t, idxs]
                pm[:ns, ei, 5] = opacity[idxs, 0]
                pm[:ns, ei, 6:9] = features_dc[idxs]
            pm[:, ei, 9] = tx * TILE
            pm[:, ei, 10] = ty * TILE
        in_maps.append({"params": pm.reshape(SLOTS, E * 12),
                        "basis": basis, "ident": ident})

    if E not in _CACHE:
        _CACHE[E] = _build_nc(E)
    nc = _CACHE[E]

    trace = bool(int(os.environ.get("GS_TRACE", "0")))
    if trace:
        _ensure_ntff_hook()
    res = bass_utils.run_bass_kernel_spmd(
        nc, in_maps, core_ids=list(range(N_CORES)), trace=trace)
    kernel.last_result = res

    img = np.zeros((T, 3, H, W), np.float32)
    for c in range(N_CORES):
        o = res.results[c]["out"].reshape(3, E, TILE, TILE)
        for ei in range(E):
            k = c * E + ei
            if k >= len(entries):
                continue
            t, ty, tx, _ = entries[k]
            img[t, :, ty * TILE:(ty + 1) * TILE, tx * TILE:(tx + 1) * TILE] = o[:, ei]
    return img


# revision 13
# speedup vs baseline: 1.1326x; 1.0062x over previous
"""GaussianImage (Cholesky) renderer on 8 trn2 NeuronCores.

Strategy: tile-parallel over the pixel grid (sharding_hint alternative 2).
The 256x256 image is cut into 32x32-pixel tiles (64/frame, 128 total for
T=2).  The host bins gaussians to tiles (pure routing: bbox intersect via a
conservative support radius; outside it exp(-sigma) underflows to 0 in
fp32), pads each tile's gaussian list to 128 slots, and hands every core 16
tile-entries with slot-ordered copies of the RAW inputs.  All math runs on
device:

  per gaussian slot : tanh / sigmoid / conic / quadratic-basis coeffs
  per tile          : sigma = lhsT(6,128)^T @ basis(6,1024)   [TensorE fp32]
                      alpha = Exp(-sigma)                     [ScalarE]
                      img   = w(128,3)^T @ alpha(128,1024)    [TensorE fp32]
                      out   = clamp(img, 0, 1)                [VectorE, fused]

Each pixel is owned by exactly one tile -> no cross-core reduction.
"""

import os
import numpy as np

T, N, H, W = 2, 512, 256, 256
TILE = 32
NT = H // TILE          # 8 tiles per axis
N_CORES = 8
SLOTS = 128
PIX = TILE * TILE       # 1024
SIGMA_CUT = 100.0       # exp(-100) ~ 4e-44: below fp32 denormal resolution

_CACHE = {}


def _build_nc(E, mm2_dtype_name="float32"):
    import concourse.bass as bass
    import concourse.mybir as mybir
    from concourse.tile import TileContext
    import bass_rust

    f32 = mybir.dt.float32
    Alu = mybir.AluOpType
    Act = mybir.ActivationFunctionType

    nc = bass.Bass("TRN2")
    params = nc.dram_tensor("params", [SLOTS, E * 12], f32, kind="ExternalInput")
    basis = nc.dram_tensor("basis", [6, PIX], f32, kind="ExternalInput")
    ident = nc.dram_tensor("ident", [SLOTS, SLOTS], f32, kind="ExternalInput")
    out = nc.dram_tensor("out", [3, E * PIX], f32, kind="ExternalOutput")

    with TileContext(nc) as tc:
        with tc.tile_pool(name="const", bufs=1) as cpool, \
             tc.tile_pool(name="work", bufs=3) as wpool, \
             tc.tile_pool(name="ps_sig", bufs=2, space="PSUM") as ps_sig_pool, \
             tc.tile_pool(name="ps_img", bufs=2, space="PSUM") as ps_img_pool:

            p3 = cpool.tile([SLOTS, E, 12], f32, tag="params")
            bt = cpool.tile([6, PIX], f32, tag="basis")
            it = cpool.tile([SLOTS, SLOTS], f32, tag="ident")
            nc.sync.dma_start(out=p3, in_=params[:].rearrange("p (e k) -> p e k", k=12))
            nc.sync.dma_start(out=bt, in_=basis[:])
            nc.sync.dma_start(out=it, in_=ident[:])

            def sc(tag):
                return cpool.tile([SLOTS, E], f32, tag=tag, name=tag)

            V = nc.vector
            S = nc.scalar

            # --- per-slot prep (all (128,E)) ---
            mx, my = sc("mx"), sc("my")
            S.activation(mx, p3[:, :, 0], Act.Tanh)
            S.activation(my, p3[:, :, 1], Act.Tanh)
            ex, ey = sc("ex"), sc("ey")
            V.scalar_tensor_tensor(out=ex, in0=mx, scalar=0.5 * W, in1=p3[:, :, 9],
                                   op0=Alu.mult, op1=Alu.subtract)
            V.scalar_tensor_tensor(out=ey, in0=my, scalar=0.5 * H, in1=p3[:, :, 10],
                                   op0=Alu.mult, op1=Alu.subtract)

            a0, a2 = sc("a0"), sc("a2")
            V.tensor_scalar_add(out=a0, in0=p3[:, :, 2], scalar1=0.5)
            V.tensor_scalar_add(out=a2, in0=p3[:, :, 4], scalar1=0.5)
            a1 = p3[:, :, 3]
            t0, t1, t2, t3 = sc("t0"), sc("t1"), sc("t2"), sc("t3")
            V.tensor_mul(out=t0, in0=a0, in1=a0)
            V.tensor_mul(out=t1, in0=a0, in1=a1)
            V.tensor_mul(out=t2, in0=a1, in1=a1)
            V.tensor_mul(out=t3, in0=a2, in1=a2)
            syy = sc("syy")
            V.tensor_add(out=syy, in0=t2, in1=t3)
            u, v, det, rdet = sc("u"), sc("v"), sc("det"), sc("rdet")
            V.tensor_mul(out=u, in0=t0, in1=syy)
            V.tensor_mul(out=v, in0=t1, in1=t1)
            V.tensor_sub(out=det, in0=u, in1=v)
            V.reciprocal(out=rdet, in_=det)
            ca, cbn, cc = sc("ca"), sc("cbn"), sc("cc")
            V.tensor_mul(out=ca, in0=syy, in1=rdet)   # conic a
            V.tensor_mul(out=cbn, in0=t1, in1=rdet)   # -conic b
            V.tensor_mul(out=cc, in0=t0, in1=rdet)    # conic c

            ct = cpool.tile([SLOTS, E, 6], f32, tag="coef")
            V.tensor_scalar_mul(out=ct[:, :, 0], in0=ca, scalar1=0.5)
            V.tensor_scalar_mul(out=ct[:, :, 1], in0=cbn, scalar1=-1.0)
            V.tensor_scalar_mul(out=ct[:, :, 2], in0=cc, scalar1=0.5)
            m1, m2 = sc("m1"), sc("m2")
            V.tensor_mul(out=m1, in0=ca, in1=ex)
            V.tensor_mul(out=m2, in0=cbn, in1=ey)
            V.tensor_sub(out=ct[:, :, 3], in0=m2, in1=m1)    # -(ca*ex + cb*ey)
            m3, m4 = sc("m3"), sc("m4")
            V.tensor_mul(out=m3, in0=cc, in1=ey)
            V.tensor_mul(out=m4, in0=cbn, in1=ex)
            V.tensor_sub(out=ct[:, :, 4], in0=m4, in1=m3)    # -(cc*ey + cb*ex)
            exx, exy, eyy = sc("exx"), sc("exy"), sc("eyy")
            V.tensor_mul(out=exx, in0=ex, in1=ex)
            V.tensor_mul(out=exy, in0=ex, in1=ey)
            V.tensor_mul(out=eyy, in0=ey, in1=ey)
            p1, p2, p3b, q = sc("p1"), sc("p2"), sc("p3b"), sc("q")
            V.tensor_mul(out=p1, in0=ct[:, :, 0], in1=exx)
            V.tensor_mul(out=p2, in0=cbn, in1=exy)
            V.tensor_mul(out=p3b, in0=ct[:, :, 2], in1=eyy)
            V.tensor_sub(out=q, in0=p1, in1=p2)
            V.tensor_add(out=ct[:, :, 5], in0=q, in1=p3b)

            osg = sc("osg")
            S.activation(osg, p3[:, :, 5], Act.Sigmoid)
            wt = cpool.tile([SLOTS, E, 3], f32, tag="w")
            S.activation(wt, p3[:, :, 6:9], Act.Sigmoid)
            f32r = mybir.dt.float32r
            wtr = cpool.tile([SLOTS, E, 3], f32r, tag="wr")
            for k in range(3):
                V.tensor_mul(out=wtr[:, :, k], in0=wt[:, :, k], in1=osg)

            # --- transpose coeffs: one PE transpose (128,6E)->(6E,128), then
            # slice per entry out of an SBUF copy ---
            tp = ps_img_pool.tile([6 * E, SLOTS], f32, tag="img", name="tp")
            nc.tensor.transpose(tp, ct.rearrange("p e k -> p (e k)"), it)
            tps = cpool.tile([6 * E, SLOTS], f32, tag="tps")
            V.tensor_copy(out=tps, in_=tp)
            lhsT = cpool.tile([6, E, SLOTS], f32, tag="lhsT")
            for e in range(E):
                nc.sync.dma_start(out=lhsT[:, e, :], in_=tps[6 * e:6 * e + 6, :])

            st = cpool.tile([3, E * PIX], f32, tag="stage")

            # --- hot loop ---
            for e in range(E):
                sig = ps_sig_pool.tile([SLOTS, PIX], f32, tag="sig")
                lh = lhsT[:, e, :]
                nc.tensor.matmul(sig[:, 0:512], lh, bt[:, 0:512], start=True, stop=True)
                nc.tensor.matmul(sig[:, 512:1024], lh, bt[:, 512:1024], start=True, stop=True)
                alpha = wpool.tile([SLOTS, PIX], f32r, tag="alpha")
                S.activation(alpha, sig, Act.Exp, scale=-1.0)
                img = ps_img_pool.tile([3, PIX], f32, tag="img")
                wre = wtr[:, e, :]
                nc.tensor.matmul(img[:, 0:512], wre, alpha[:, 0:512], start=True, stop=True)
                nc.tensor.matmul(img[:, 512:1024], wre, alpha[:, 512:1024], start=True, stop=True)
                V.tensor_scalar(out=st[:, e * PIX:(e + 1) * PIX], in0=img,
                                scalar1=0.0, scalar2=1.0, op0=Alu.max, op1=Alu.min)
                nc.sync.dma_start(out=out[:, e * PIX:(e + 1) * PIX],
                                  in_=st[:, e * PIX:(e + 1) * PIX])

    bass_rust.generate_event_semaphores(nc)
    return nc


def _bin_entries(xyz, cholesky):
    """Host-side routing: which gaussians overlap which 32x32 tile."""
    means = np.tanh(xyz.astype(np.float64))
    cx = 0.5 * W * (means[..., 0] + 1.0)
    cy = 0.5 * H * (means[..., 1] + 1.0)
    chol = cholesky.astype(np.float64) + np.array([0.5, 0.0, 0.5])
    l0, l1, l2 = chol[..., 0], chol[..., 1], chol[..., 2]
    sxx, sxy, syy = l0 * l0, l0 * l1, l1 * l1 + l2 * l2
    tr, det = sxx + syy, sxx * syy - sxy * sxy
    lam = tr / 2 + np.sqrt(np.maximum(tr * tr / 4 - det, 0.0))
    r = np.sqrt(2.0 * SIGMA_CUT * np.maximum(lam, 0.0)) + 1.0

    entries = []  # (frame, ty, tx, index-list)
    for t in range(T):
        x0 = np.clip(((cx[t] - r[t]) // TILE).astype(int), 0, NT - 1)
        x1 = np.clip(((cx[t] + r[t]) // TILE).astype(int), 0, NT - 1)
        y0 = np.clip(((cy[t] - r[t]) // TILE).astype(int), 0, NT - 1)
        y1 = np.clip(((cy[t] + r[t]) // TILE).astype(int), 0, NT - 1)
        buckets = [[[] for _ in range(NT)] for _ in range(NT)]
        for n in range(N):
            for ty in range(y0[n], y1[n] + 1):
                for tx in range(x0[n], x1[n] + 1):
                    buckets[ty][tx].append(n)
        for ty in range(NT):
            for tx in range(NT):
                assert len(buckets[ty][tx]) <= SLOTS, "tile overflow: >128 gaussians"
                entries.append((t, ty, tx, buckets[ty][tx]))
    return entries


def _ensure_ntff_hook():
    """Provide antenv.axon_hooks (missing in this image) so trace=True works."""
    import sys, types, ctypes, contextlib
    if "antenv.axon_hooks" in sys.modules:
        return
    so_path = "/opt/axon/libaxon_pjrt.so"
    if not os.path.exists(so_path):
        return
    lib = ctypes.CDLL(so_path)
    if not hasattr(lib, "axon_start_nrt_profile"):
        return
    lib.axon_start_nrt_profile.argtypes = [ctypes.POINTER(ctypes.c_int64), ctypes.c_size_t]
    lib.axon_start_nrt_profile.restype = ctypes.c_int64
    lib.axon_stop_nrt_profile.argtypes = [ctypes.c_char_p]
    lib.axon_stop_nrt_profile.restype = ctypes.c_int64

    @contextlib.contextmanager
    def _hook(output_dir, device_ids):
        import jax
        jax.devices()
        if device_ids:
            ids = (ctypes.c_int64 * len(device_ids))(*device_ids)
            rc = lib.axon_start_nrt_profile(ids, len(device_ids))
        else:
            rc = lib.axon_start_nrt_profile(None, 0)
        if rc != 0:
            raise RuntimeError(f"axon_start_nrt_profile rc={rc}")
        try:
            yield
        finally:
            n = lib.axon_stop_nrt_profile(str(output_dir).encode())
            print(f"profile: {n} file(s) written to {output_dir}")

    mod = types.ModuleType("antenv.axon_hooks")
    mod.get_axon_ntff_profile_hook = lambda: _hook
    mod.set_axon_ntff_profile_hook = lambda h: None
    sys.modules["antenv.axon_hooks"] = mod


def kernel(xyz, cholesky, opacity, features_dc):
    from concourse import bass_utils

    xyz = np.asarray(xyz, np.float32)
    cholesky = np.asarray(cholesky, np.float32)
    opacity = np.asarray(opacity, np.float32)
    features_dc = np.asarray(features_dc, np.float32)

    entries = _bin_entries(xyz, cholesky)
    E = (len(entries) + N_CORES - 1) // N_CORES

    # per-core packed params: (128, E, 12) -> flat (128, E*12)
    in_maps = []
    gx = np.arange(PIX, dtype=np.float32) % TILE
    gy = np.arange(PIX, dtype=np.float32) // TILE
    basis = np.stack([gx * gx, gx * gy, gy * gy, gx, gy, np.ones(PIX, np.float32)]).astype(np.float32)
    ident = np.eye(SLOTS, dtype=np.float32)
    for c in range(N_CORES):
        pm = np.zeros((SLOTS, E, 12), np.float32)
        pm[:, :, 5] = -100.0  # dummy slots: sigmoid(opacity) ~ 0
        for ei in range(E):
            k = c * E + ei
            if k >= len(entries):
                continue
            t, ty, tx, idxs = entries[k]
            ns = len(idxs)
            if ns:
                idxs = np.asarray(idxs)
                pm[:ns, ei, 0:2] = xyz[t, idxs]
                pm[:ns, ei, 2:5] = cholesky[t, idxs]
                pm[:ns, ei, 5] = opacity[idxs, 0]
                pm[:ns, ei, 6:9] = features_dc[idxs]
            pm[:, ei, 9] = tx * TILE - 0.5 * W
            pm[:, ei, 10] = ty * TILE - 0.5 * H
        in_maps.append({"params": pm.reshape(SLOTS, E * 12),
                        "basis": basis, "ident": ident})

    if E not in _CACHE:
        _CACHE[E] = _build_nc(E)
    nc = _CACHE[E]

    trace = bool(int(os.environ.get("GS_TRACE", "0")))
    if trace:
        _ensure_ntff_hook()
    res = bass_utils.run_bass_kernel_spmd(
        nc, in_maps, core_ids=list(range(N_CORES)), trace=trace)
    kernel.last_result = res

    img = np.zeros((T, 3, H, W), np.float32)
    for c in range(N_CORES):
        o = res.results[c]["out"].reshape(3, E, TILE, TILE)
        for ei in range(E):
            k = c * E + ei
            if k >= len(entries):
                continue
            t, ty, tx, _ = entries[k]
            img[t, :, ty * TILE:(ty + 1) * TILE, tx * TILE:(tx + 1) * TILE] = o[:, ei]
    return img


# revision 15
# speedup vs baseline: 1.1331x; 1.0004x over previous
"""GaussianImage (Cholesky) renderer on 8 trn2 NeuronCores.

Strategy: tile-parallel over the pixel grid (sharding_hint alternative 2).
The 256x256 image is cut into 32x32-pixel tiles (64/frame, 128 total for
T=2).  The host bins gaussians to tiles (pure routing: bbox intersect via a
conservative support radius; outside it exp(-sigma) underflows to 0 in
fp32), pads each tile's gaussian list to 128 slots, and hands every core 16
tile-entries with slot-ordered copies of the RAW inputs.  All math runs on
device:

  per gaussian slot : tanh / sigmoid / conic / quadratic-basis coeffs
  per tile          : sigma = lhsT(6,128)^T @ basis(6,1024)   [TensorE fp32]
                      alpha = Exp(-sigma)                     [ScalarE]
                      img   = w(128,3)^T @ alpha(128,1024)    [TensorE fp32]
                      out   = clamp(img, 0, 1)                [VectorE, fused]

Each pixel is owned by exactly one tile -> no cross-core reduction.
"""

import os
import numpy as np

T, N, H, W = 2, 512, 256, 256
TILE = 32
NT = H // TILE          # 8 tiles per axis
N_CORES = 8
SLOTS = 128
PIX = TILE * TILE       # 1024
SIGMA_CUT = 100.0       # exp(-100) ~ 4e-44: below fp32 denormal resolution

_CACHE = {}


def _build_nc(E, mm2_dtype_name="float32"):
    import concourse.bass as bass
    import concourse.mybir as mybir
    from concourse.tile import TileContext
    import bass_rust

    f32 = mybir.dt.float32
    Alu = mybir.AluOpType
    Act = mybir.ActivationFunctionType

    nc = bass.Bass("TRN2")
    params = nc.dram_tensor("params", [SLOTS, E * 12], f32, kind="ExternalInput")
    basis = nc.dram_tensor("basis", [6, PIX], f32, kind="ExternalInput")
    ident = nc.dram_tensor("ident", [SLOTS, SLOTS], f32, kind="ExternalInput")
    out = nc.dram_tensor("out", [3, E * PIX], f32, kind="ExternalOutput")

    with TileContext(nc) as tc:
        with tc.tile_pool(name="const", bufs=1) as cpool, \
             tc.tile_pool(name="work", bufs=3) as wpool, \
             tc.tile_pool(name="ps_sig", bufs=2, space="PSUM") as ps_sig_pool, \
             tc.tile_pool(name="ps_img", bufs=2, space="PSUM") as ps_img_pool:

            p3 = cpool.tile([SLOTS, E, 12], f32, tag="params")
            bt = cpool.tile([6, PIX], f32, tag="basis")
            it = cpool.tile([SLOTS, SLOTS], f32, tag="ident")
            nc.sync.dma_start(out=p3, in_=params[:].rearrange("p (e k) -> p e k", k=12))
            nc.sync.dma_start(out=bt, in_=basis[:])
            nc.sync.dma_start(out=it, in_=ident[:])

            def sc(tag):
                return cpool.tile([SLOTS, EH], f32, tag=tag, name=tag)

            V = nc.vector
            S = nc.scalar
            EH = E // 2 if E % 2 == 0 else E
            NHALF = E // EH

            ct = cpool.tile([SLOTS, E, 6], f32, tag="coef")
            wt = cpool.tile([SLOTS, E, 3], f32, tag="w")
            f32r = mybir.dt.float32r
            wtr = cpool.tile([SLOTS, E, 3], f32r, tag="wr")
            lhsT = cpool.tile([6, E, SLOTS], f32, tag="lhsT")

            for h in range(NHALF):
                es = slice(h * EH, (h + 1) * EH)
                def sc(tag, h=h):
                    return cpool.tile([SLOTS, EH], f32, tag=f"{tag}h{h}", name=f"{tag}h{h}")
                p3h = p3[:, es, :]
                cth = ct[:, es, :]
                mx, my = sc("mx"), sc("my")
                S.activation(mx, p3h[:, :, 0], Act.Tanh)
                S.activation(my, p3h[:, :, 1], Act.Tanh)
                ex, ey = sc("ex"), sc("ey")
                V.scalar_tensor_tensor(out=ex, in0=mx, scalar=0.5 * W, in1=p3h[:, :, 9],
                                       op0=Alu.mult, op1=Alu.subtract)
                V.scalar_tensor_tensor(out=ey, in0=my, scalar=0.5 * H, in1=p3h[:, :, 10],
                                       op0=Alu.mult, op1=Alu.subtract)
                a0, a2 = sc("a0"), sc("a2")
                V.tensor_scalar_add(out=a0, in0=p3h[:, :, 2], scalar1=0.5)
                V.tensor_scalar_add(out=a2, in0=p3h[:, :, 4], scalar1=0.5)
                a1 = p3h[:, :, 3]
                t0, t1, t2, t3 = sc("t0"), sc("t1"), sc("t2"), sc("t3")
                V.tensor_mul(out=t0, in0=a0, in1=a0)
                V.tensor_mul(out=t1, in0=a0, in1=a1)
                V.tensor_mul(out=t2, in0=a1, in1=a1)
                V.tensor_mul(out=t3, in0=a2, in1=a2)
                syy = sc("syy")
                V.tensor_add(out=syy, in0=t2, in1=t3)
                u, v, det, rdet = sc("u"), sc("v"), sc("det"), sc("rdet")
                V.tensor_mul(out=u, in0=t0, in1=syy)
                V.tensor_mul(out=v, in0=t1, in1=t1)
                V.tensor_sub(out=det, in0=u, in1=v)
                V.reciprocal(out=rdet, in_=det)
                ca, cbn, cc = sc("ca"), sc("cbn"), sc("cc")
                V.tensor_mul(out=ca, in0=syy, in1=rdet)
                V.tensor_mul(out=cbn, in0=t1, in1=rdet)
                V.tensor_mul(out=cc, in0=t0, in1=rdet)
                V.tensor_scalar_mul(out=cth[:, :, 0], in0=ca, scalar1=0.5)
                V.tensor_scalar_mul(out=cth[:, :, 1], in0=cbn, scalar1=-1.0)
                V.tensor_scalar_mul(out=cth[:, :, 2], in0=cc, scalar1=0.5)
                m1, m2 = sc("m1"), sc("m2")
                V.tensor_mul(out=m1, in0=ca, in1=ex)
                V.tensor_mul(out=m2, in0=cbn, in1=ey)
                V.tensor_sub(out=cth[:, :, 3], in0=m2, in1=m1)
                m3, m4 = sc("m3"), sc("m4")
                V.tensor_mul(out=m3, in0=cc, in1=ey)
                V.tensor_mul(out=m4, in0=cbn, in1=ex)
                V.tensor_sub(out=cth[:, :, 4], in0=m4, in1=m3)
                exx, exy, eyy = sc("exx"), sc("exy"), sc("eyy")
                V.tensor_mul(out=exx, in0=ex, in1=ex)
                V.tensor_mul(out=exy, in0=ex, in1=ey)
                V.tensor_mul(out=eyy, in0=ey, in1=ey)
                p1, p2, p3b, q = sc("p1"), sc("p2"), sc("p3b"), sc("q")
                V.tensor_mul(out=p1, in0=cth[:, :, 0], in1=exx)
                V.tensor_mul(out=p2, in0=cbn, in1=exy)
                V.tensor_mul(out=p3b, in0=cth[:, :, 2], in1=eyy)
                V.tensor_sub(out=q, in0=p1, in1=p2)
                V.tensor_add(out=cth[:, :, 5], in0=q, in1=p3b)
                osg = sc("osg")
                S.activation(osg, p3h[:, :, 5], Act.Sigmoid)
                S.activation(wt[:, es, :], p3h[:, :, 6:9], Act.Sigmoid)
                for k in range(3):
                    V.tensor_mul(out=wtr[:, es, k], in0=wt[:, es, k], in1=osg)
                tp = ps_img_pool.tile([6 * EH, SLOTS], f32, tag="img", name=f"tp{h}")
                nc.tensor.transpose(tp, cth.rearrange("p e k -> p (e k)"), it)
                tps = cpool.tile([6 * EH, SLOTS], f32, tag=f"tpsh{h}", name=f"tpsh{h}")
                V.tensor_copy(out=tps, in_=tp)
                for j in range(EH):
                    nc.sync.dma_start(out=lhsT[:, h * EH + j, :],
                                      in_=tps[6 * j:6 * j + 6, :])

            st = cpool.tile([3, E * PIX], f32, tag="stage")

            # --- hot loop ---
            for e in range(E):
                sig = ps_sig_pool.tile([SLOTS, PIX], f32, tag="sig")
                lh = lhsT[:, e, :]
                nc.tensor.matmul(sig[:, 0:512], lh, bt[:, 0:512], start=True, stop=True)
                nc.tensor.matmul(sig[:, 512:1024], lh, bt[:, 512:1024], start=True, stop=True)
                alpha = wpool.tile([SLOTS, PIX], f32r, tag="alpha")
                S.activation(alpha, sig, Act.Exp, scale=-1.0)
                img = ps_img_pool.tile([3, PIX], f32, tag="img")
                wre = wtr[:, e, :]
                nc.tensor.matmul(img[:, 0:512], wre, alpha[:, 0:512], start=True, stop=True)
                nc.tensor.matmul(img[:, 512:1024], wre, alpha[:, 512:1024], start=True, stop=True)
                V.tensor_scalar(out=st[:, e * PIX:(e + 1) * PIX], in0=img,
                                scalar1=0.0, scalar2=1.0, op0=Alu.max, op1=Alu.min)
                nc.sync.dma_start(out=out[:, e * PIX:(e + 1) * PIX],
                                  in_=st[:, e * PIX:(e + 1) * PIX])

    bass_rust.generate_event_semaphores(nc)
    return nc


def _bin_entries(xyz, cholesky):
    """Host-side routing: which gaussians overlap which 32x32 tile."""
    means = np.tanh(xyz.astype(np.float64))
    cx = 0.5 * W * (means[..., 0] + 1.0)
    cy = 0.5 * H * (means[..., 1] + 1.0)
    chol = cholesky.astype(np.float64) + np.array([0.5, 0.0, 0.5])
    l0, l1, l2 = chol[..., 0], chol[..., 1], chol[..., 2]
    sxx, sxy, syy = l0 * l0, l0 * l1, l1 * l1 + l2 * l2
    tr, det = sxx + syy, sxx * syy - sxy * sxy
    lam = tr / 2 + np.sqrt(np.maximum(tr * tr / 4 - det, 0.0))
    r = np.sqrt(2.0 * SIGMA_CUT * np.maximum(lam, 0.0)) + 1.0

    entries = []  # (frame, ty, tx, index-list)
    for t in range(T):
        x0 = np.clip(((cx[t] - r[t]) // TILE).astype(int), 0, NT - 1)
        x1 = np.clip(((cx[t] + r[t]) // TILE).astype(int), 0, NT - 1)
        y0 = np.clip(((cy[t] - r[t]) // TILE).astype(int), 0, NT - 1)
        y1 = np.clip(((cy[t] + r[t]) // TILE).astype(int), 0, NT - 1)
        buckets = [[[] for _ in range(NT)] for _ in range(NT)]
        for n in range(N):
            for ty in range(y0[n], y1[n] + 1):
                for tx in range(x0[n], x1[n] + 1):
                    buckets[ty][tx].append(n)
        for ty in range(NT):
            for tx in range(NT):
                assert len(buckets[ty][tx]) <= SLOTS, "tile overflow: >128 gaussians"
                entries.append((t, ty, tx, buckets[ty][tx]))
    return entries


def _ensure_ntff_hook():
    """Provide antenv.axon_hooks (missing in this image) so trace=True works."""
    import sys, types, ctypes, contextlib
    if "antenv.axon_hooks" in sys.modules:
        return
    so_path = "/opt/axon/libaxon_pjrt.so"
    if not os.path.exists(so_path):
        return
    lib = ctypes.CDLL(so_path)
    if not hasattr(lib, "axon_start_nrt_profile"):
        return
    lib.axon_start_nrt_profile.argtypes = [ctypes.POINTER(ctypes.c_int64), ctypes.c_size_t]
    lib.axon_start_nrt_profile.restype = ctypes.c_int64
    lib.axon_stop_nrt_profile.argtypes = [ctypes.c_char_p]
    lib.axon_stop_nrt_profile.restype = ctypes.c_int64

    @contextlib.contextmanager
    def _hook(output_dir, device_ids):
        import jax
        jax.devices()
        if device_ids:
            ids = (ctypes.c_int64 * len(device_ids))(*device_ids)
            rc = lib.axon_start_nrt_profile(ids, len(device_ids))
        else:
            rc = lib.axon_start_nrt_profile(None, 0)
        if rc != 0:
            raise RuntimeError(f"axon_start_nrt_profile rc={rc}")
        try:
            yield
        finally:
            n = lib.axon_stop_nrt_profile(str(output_dir).encode())
            print(f"profile: {n} file(s) written to {output_dir}")

    mod = types.ModuleType("antenv.axon_hooks")
    mod.get_axon_ntff_profile_hook = lambda: _hook
    mod.set_axon_ntff_profile_hook = lambda h: None
    sys.modules["antenv.axon_hooks"] = mod


def kernel(xyz, cholesky, opacity, features_dc):
    from concourse import bass_utils

    xyz = np.asarray(xyz, np.float32)
    cholesky = np.asarray(cholesky, np.float32)
    opacity = np.asarray(opacity, np.float32)
    features_dc = np.asarray(features_dc, np.float32)

    entries = _bin_entries(xyz, cholesky)
    E = (len(entries) + N_CORES - 1) // N_CORES

    # per-core packed params: (128, E, 12) -> flat (128, E*12)
    in_maps = []
    gx = np.arange(PIX, dtype=np.float32) % TILE
    gy = np.arange(PIX, dtype=np.float32) // TILE
    basis = np.stack([gx * gx, gx * gy, gy * gy, gx, gy, np.ones(PIX, np.float32)]).astype(np.float32)
    ident = np.eye(SLOTS, dtype=np.float32)
    for c in range(N_CORES):
        pm = np.zeros((SLOTS, E, 12), np.float32)
        pm[:, :, 5] = -100.0  # dummy slots: sigmoid(opacity) ~ 0
        for ei in range(E):
            k = c * E + ei
            if k >= len(entries):
                continue
            t, ty, tx, idxs = entries[k]
            ns = len(idxs)
            if ns:
                idxs = np.asarray(idxs)
                pm[:ns, ei, 0:2] = xyz[t, idxs]
                pm[:ns, ei, 2:5] = cholesky[t, idxs]
                pm[:ns, ei, 5] = opacity[idxs, 0]
                pm[:ns, ei, 6:9] = features_dc[idxs]
            pm[:, ei, 9] = tx * TILE - 0.5 * W
            pm[:, ei, 10] = ty * TILE - 0.5 * H
        in_maps.append({"params": pm.reshape(SLOTS, E * 12),
                        "basis": basis, "ident": ident})

    if E not in _CACHE:
        _CACHE[E] = _build_nc(E)
    nc = _CACHE[E]

    trace = bool(int(os.environ.get("GS_TRACE", "0")))
    if trace:
        _ensure_ntff_hook()
    res = bass_utils.run_bass_kernel_spmd(
        nc, in_maps, core_ids=list(range(N_CORES)), trace=trace)
    kernel.last_result = res

    img = np.zeros((T, 3, H, W), np.float32)
    for c in range(N_CORES):
        o = res.results[c]["out"].reshape(3, E, TILE, TILE)
        for ei in range(E):
            k = c * E + ei
            if k >= len(entries):
                continue
            t, ty, tx, _ = entries[k]
            img[t, :, ty * TILE:(ty + 1) * TILE, tx * TILE:(tx + 1) * TILE] = o[:, ei]
    return img


# revision 16
# speedup vs baseline: 1.1353x; 1.0020x over previous
"""GaussianImage (Cholesky) renderer on 8 trn2 NeuronCores.

Strategy: tile-parallel over the pixel grid (sharding_hint alternative 2).
The 256x256 image is cut into 32x32-pixel tiles (64/frame, 128 total for
T=2).  The host bins gaussians to tiles (pure routing: bbox intersect via a
conservative support radius; outside it exp(-sigma) underflows to 0 in
fp32), pads each tile's gaussian list to 128 slots, and hands every core 16
tile-entries with slot-ordered copies of the RAW inputs.  All math runs on
device:

  per gaussian slot : tanh / sigmoid / conic / quadratic-basis coeffs
  per tile          : sigma = lhsT(6,128)^T @ basis(6,1024)   [TensorE fp32]
                      alpha = Exp(-sigma)                     [ScalarE]
                      img   = w(128,3)^T @ alpha(128,1024)    [TensorE fp32]
                      out   = clamp(img, 0, 1)                [VectorE, fused]

Each pixel is owned by exactly one tile -> no cross-core reduction.
"""

import os
import numpy as np

T, N, H, W = 2, 512, 256, 256
TILE = 32
NT = H // TILE          # 8 tiles per axis
N_CORES = 8
SLOTS = 128
PIX = TILE * TILE       # 1024
SIGMA_CUT = 100.0       # exp(-100) ~ 4e-44: below fp32 denormal resolution

_CACHE = {}


def _build_nc(E, mm2_dtype_name="float32"):
    import concourse.bass as bass
    import concourse.mybir as mybir
    from concourse.tile import TileContext
    import bass_rust

    f32 = mybir.dt.float32
    Alu = mybir.AluOpType
    Act = mybir.ActivationFunctionType

    nc = bass.Bass("TRN2")
    params = nc.dram_tensor("params", [SLOTS, E * 12], f32, kind="ExternalInput")
    basis = nc.dram_tensor("basis", [6, PIX], f32, kind="ExternalInput")
    ident = nc.dram_tensor("ident", [SLOTS, SLOTS], f32, kind="ExternalInput")
    out = nc.dram_tensor("out", [3, E * PIX], f32, kind="ExternalOutput")

    with TileContext(nc) as tc:
        with tc.tile_pool(name="const", bufs=1) as cpool, \
             tc.tile_pool(name="work", bufs=3) as wpool, \
             tc.tile_pool(name="ps_sig", bufs=2, space="PSUM") as ps_sig_pool, \
             tc.tile_pool(name="ps_img", bufs=2, space="PSUM") as ps_img_pool:

            p3 = cpool.tile([SLOTS, E, 12], f32, tag="params")
            bt = cpool.tile([6, PIX], f32, tag="basis")
            it = cpool.tile([SLOTS, SLOTS], f32, tag="ident")
            nc.sync.dma_start(out=p3, in_=params[:].rearrange("p (e k) -> p e k", k=12))
            nc.sync.dma_start(out=bt, in_=basis[:])
            nc.sync.dma_start(out=it, in_=ident[:])

            def sc(tag):
                return cpool.tile([SLOTS, EH], f32, tag=tag, name=tag)

            V = nc.vector
            S = nc.scalar
            EH = E // 2 if E % 2 == 0 else E
            NHALF = E // EH

            ct = cpool.tile([SLOTS, E, 6], f32, tag="coef")
            wt = cpool.tile([SLOTS, E, 3], f32, tag="w")
            f32r = mybir.dt.float32r
            wtr = cpool.tile([SLOTS, E, 3], f32r, tag="wr")
            lhsT = cpool.tile([6, E, SLOTS], f32, tag="lhsT")

            # warm the sigmoid/tanh ACT table set while the params DMA is in
            # flight: the table load (~2.7us) otherwise serializes after it
            warm = cpool.tile([SLOTS, 1], f32, tag="warm")
            nc.gpsimd.memset(warm, 0.0)
            S.activation(warm, warm, Act.Sigmoid)

            for h in range(NHALF):
                es = slice(h * EH, (h + 1) * EH)
                def sc(tag, h=h):
                    return cpool.tile([SLOTS, EH], f32, tag=f"{tag}h{h}", name=f"{tag}h{h}")
                p3h = p3[:, es, :]
                cth = ct[:, es, :]
                mx, my = sc("mx"), sc("my")
                S.activation(mx, p3h[:, :, 0], Act.Tanh)
                S.activation(my, p3h[:, :, 1], Act.Tanh)
                ex, ey = sc("ex"), sc("ey")
                V.scalar_tensor_tensor(out=ex, in0=mx, scalar=0.5 * W, in1=p3h[:, :, 9],
                                       op0=Alu.mult, op1=Alu.subtract)
                V.scalar_tensor_tensor(out=ey, in0=my, scalar=0.5 * H, in1=p3h[:, :, 10],
                                       op0=Alu.mult, op1=Alu.subtract)
                a0, a2 = sc("a0"), sc("a2")
                V.tensor_scalar_add(out=a0, in0=p3h[:, :, 2], scalar1=0.5)
                V.tensor_scalar_add(out=a2, in0=p3h[:, :, 4], scalar1=0.5)
                a1 = p3h[:, :, 3]
                t0, t1, t2, t3 = sc("t0"), sc("t1"), sc("t2"), sc("t3")
                V.tensor_mul(out=t0, in0=a0, in1=a0)
                V.tensor_mul(out=t1, in0=a0, in1=a1)
                V.tensor_mul(out=t2, in0=a1, in1=a1)
                V.tensor_mul(out=t3, in0=a2, in1=a2)
                syy = sc("syy")
                V.tensor_add(out=syy, in0=t2, in1=t3)
                u, v, det, rdet = sc("u"), sc("v"), sc("det"), sc("rdet")
                V.tensor_mul(out=u, in0=t0, in1=syy)
                V.tensor_mul(out=v, in0=t1, in1=t1)
                V.tensor_sub(out=det, in0=u, in1=v)
                V.reciprocal(out=rdet, in_=det)
                ca, cbn, cc = sc("ca"), sc("cbn"), sc("cc")
                V.tensor_mul(out=ca, in0=syy, in1=rdet)
                V.tensor_mul(out=cbn, in0=t1, in1=rdet)
                V.tensor_mul(out=cc, in0=t0, in1=rdet)
                V.tensor_scalar_mul(out=cth[:, :, 0], in0=ca, scalar1=0.5)
                V.tensor_scalar_mul(out=cth[:, :, 1], in0=cbn, scalar1=-1.0)
                V.tensor_scalar_mul(out=cth[:, :, 2], in0=cc, scalar1=0.5)
                m1, m2 = sc("m1"), sc("m2")
                V.tensor_mul(out=m1, in0=ca, in1=ex)
                V.tensor_mul(out=m2, in0=cbn, in1=ey)
                V.tensor_sub(out=cth[:, :, 3], in0=m2, in1=m1)
                m3, m4 = sc("m3"), sc("m4")
                V.tensor_mul(out=m3, in0=cc, in1=ey)
                V.tensor_mul(out=m4, in0=cbn, in1=ex)
                V.tensor_sub(out=cth[:, :, 4], in0=m4, in1=m3)
                exx, exy, eyy = sc("exx"), sc("exy"), sc("eyy")
                V.tensor_mul(out=exx, in0=ex, in1=ex)
                V.tensor_mul(out=exy, in0=ex, in1=ey)
                V.tensor_mul(out=eyy, in0=ey, in1=ey)
                p1, p2, p3b, q = sc("p1"), sc("p2"), sc("p3b"), sc("q")
                V.tensor_mul(out=p1, in0=cth[:, :, 0], in1=exx)
                V.tensor_mul(out=p2, in0=cbn, in1=exy)
                V.tensor_mul(out=p3b, in0=cth[:, :, 2], in1=eyy)
                V.tensor_sub(out=q, in0=p1, in1=p2)
                V.tensor_add(out=cth[:, :, 5], in0=q, in1=p3b)
                osg = sc("osg")
                S.activation(osg, p3h[:, :, 5], Act.Sigmoid)
                S.activation(wt[:, es, :], p3h[:, :, 6:9], Act.Sigmoid)
                for k in range(3):
                    V.tensor_mul(out=wtr[:, es, k], in0=wt[:, es, k], in1=osg)
                tp = ps_img_pool.tile([6 * EH, SLOTS], f32, tag="img", name=f"tp{h}")
                nc.tensor.transpose(tp, cth.rearrange("p e k -> p (e k)"), it)
                tps = cpool.tile([6 * EH, SLOTS], f32, tag=f"tpsh{h}", name=f"tpsh{h}")
                V.tensor_copy(out=tps, in_=tp)
                for j in range(EH):
                    nc.sync.dma_start(out=lhsT[:, h * EH + j, :],
                                      in_=tps[6 * j:6 * j + 6, :])

            st = cpool.tile([3, E * PIX], f32, tag="stage")

            # --- hot loop ---
            for e in range(E):
                sig = ps_sig_pool.tile([SLOTS, PIX], f32, tag="sig")
                lh = lhsT[:, e, :]
                nc.tensor.matmul(sig[:, 0:512], lh, bt[:, 0:512], start=True, stop=True)
                nc.tensor.matmul(sig[:, 512:1024], lh, bt[:, 512:1024], start=True, stop=True)
                alpha = wpool.tile([SLOTS, PIX], f32r, tag="alpha")
                S.activation(alpha, sig, Act.Exp, scale=-1.0)
                img = ps_img_pool.tile([3, PIX], f32, tag="img")
                wre = wtr[:, e, :]
                nc.tensor.matmul(img[:, 0:512], wre, alpha[:, 0:512], start=True, stop=True)
                nc.tensor.matmul(img[:, 512:1024], wre, alpha[:, 512:1024], start=True, stop=True)
                V.tensor_scalar(out=st[:, e * PIX:(e + 1) * PIX], in0=img,
                                scalar1=0.0, scalar2=1.0, op0=Alu.max, op1=Alu.min)
                nc.sync.dma_start(out=out[:, e * PIX:(e + 1) * PIX],
                                  in_=st[:, e * PIX:(e + 1) * PIX])

    bass_rust.generate_event_semaphores(nc)
    return nc


def _bin_entries(xyz, cholesky):
    """Host-side routing: which gaussians overlap which 32x32 tile."""
    means = np.tanh(xyz.astype(np.float64))
    cx = 0.5 * W * (means[..., 0] + 1.0)
    cy = 0.5 * H * (means[..., 1] + 1.0)
    chol = cholesky.astype(np.float64) + np.array([0.5, 0.0, 0.5])
    l0, l1, l2 = chol[..., 0], chol[..., 1], chol[..., 2]
    sxx, sxy, syy = l0 * l0, l0 * l1, l1 * l1 + l2 * l2
    tr, det = sxx + syy, sxx * syy - sxy * sxy
    lam = tr / 2 + np.sqrt(np.maximum(tr * tr / 4 - det, 0.0))
    r = np.sqrt(2.0 * SIGMA_CUT * np.maximum(lam, 0.0)) + 1.0

    entries = []  # (frame, ty, tx, index-list)
    for t in range(T):
        x0 = np.clip(((cx[t] - r[t]) // TILE).astype(int), 0, NT - 1)
        x1 = np.clip(((cx[t] + r[t]) // TILE).astype(int), 0, NT - 1)
        y0 = np.clip(((cy[t] - r[t]) // TILE).astype(int), 0, NT - 1)
        y1 = np.clip(((cy[t] + r[t]) // TILE).astype(int), 0, NT - 1)
        buckets = [[[] for _ in range(NT)] for _ in range(NT)]
        for n in range(N):
            for ty in range(y0[n], y1[n] + 1):
                for tx in range(x0[n], x1[n] + 1):
                    buckets[ty][tx].append(n)
        for ty in range(NT):
            for tx in range(NT):
                assert len(buckets[ty][tx]) <= SLOTS, "tile overflow: >128 gaussians"
                entries.append((t, ty, tx, buckets[ty][tx]))
    return entries


def _ensure_ntff_hook():
    """Provide antenv.axon_hooks (missing in this image) so trace=True works."""
    import sys, types, ctypes, contextlib
    if "antenv.axon_hooks" in sys.modules:
        return
    so_path = "/opt/axon/libaxon_pjrt.so"
    if not os.path.exists(so_path):
        return
    lib = ctypes.CDLL(so_path)
    if not hasattr(lib, "axon_start_nrt_profile"):
        return
    lib.axon_start_nrt_profile.argtypes = [ctypes.POINTER(ctypes.c_int64), ctypes.c_size_t]
    lib.axon_start_nrt_profile.restype = ctypes.c_int64
    lib.axon_stop_nrt_profile.argtypes = [ctypes.c_char_p]
    lib.axon_stop_nrt_profile.restype = ctypes.c_int64

    @contextlib.contextmanager
    def _hook(output_dir, device_ids):
        import jax
        jax.devices()
        if device_ids:
            ids = (ctypes.c_int64 * len(device_ids))(*device_ids)
            rc = lib.axon_start_nrt_profile(ids, len(device_ids))
        else:
            rc = lib.axon_start_nrt_profile(None, 0)
        if rc != 0:
            raise RuntimeError(f"axon_start_nrt_profile rc={rc}")
        try:
            yield
        finally:
            n = lib.axon_stop_nrt_profile(str(output_dir).encode())
            print(f"profile: {n} file(s) written to {output_dir}")

    mod = types.ModuleType("antenv.axon_hooks")
    mod.get_axon_ntff_profile_hook = lambda: _hook
    mod.set_axon_ntff_profile_hook = lambda h: None
    sys.modules["antenv.axon_hooks"] = mod


def kernel(xyz, cholesky, opacity, features_dc):
    from concourse import bass_utils

    xyz = np.asarray(xyz, np.float32)
    cholesky = np.asarray(cholesky, np.float32)
    opacity = np.asarray(opacity, np.float32)
    features_dc = np.asarray(features_dc, np.float32)

    entries = _bin_entries(xyz, cholesky)
    E = (len(entries) + N_CORES - 1) // N_CORES

    # per-core packed params: (128, E, 12) -> flat (128, E*12)
    in_maps = []
    gx = np.arange(PIX, dtype=np.float32) % TILE
    gy = np.arange(PIX, dtype=np.float32) // TILE
    basis = np.stack([gx * gx, gx * gy, gy * gy, gx, gy, np.ones(PIX, np.float32)]).astype(np.float32)
    ident = np.eye(SLOTS, dtype=np.float32)
    for c in range(N_CORES):
        pm = np.zeros((SLOTS, E, 12), np.float32)
        pm[:, :, 5] = -100.0  # dummy slots: sigmoid(opacity) ~ 0
        for ei in range(E):
            k = c * E + ei
            if k >= len(entries):
                continue
            t, ty, tx, idxs = entries[k]
            ns = len(idxs)
            if ns:
                idxs = np.asarray(idxs)
                pm[:ns, ei, 0:2] = xyz[t, idxs]
                pm[:ns, ei, 2:5] = cholesky[t, idxs]
                pm[:ns, ei, 5] = opacity[idxs, 0]
                pm[:ns, ei, 6:9] = features_dc[idxs]
            pm[:, ei, 9] = tx * TILE - 0.5 * W
            pm[:, ei, 10] = ty * TILE - 0.5 * H
        in_maps.append({"params": pm.reshape(SLOTS, E * 12),
                        "basis": basis, "ident": ident})

    if E not in _CACHE:
        _CACHE[E] = _build_nc(E)
    nc = _CACHE[E]

    trace = bool(int(os.environ.get("GS_TRACE", "0")))
    if trace:
        _ensure_ntff_hook()
    res = bass_utils.run_bass_kernel_spmd(
        nc, in_maps, core_ids=list(range(N_CORES)), trace=trace)
    kernel.last_result = res

    img = np.zeros((T, 3, H, W), np.float32)
    for c in range(N_CORES):
        o = res.results[c]["out"].reshape(3, E, TILE, TILE)
        for ei in range(E):
            k = c * E + ei
            if k >= len(entries):
                continue
            t, ty, tx, _ = entries[k]
            img[t, :, ty * TILE:(ty + 1) * TILE, tx * TILE:(tx + 1) * TILE] = o[:, ei]
    return img
